# revision 22
# baseline (speedup 1.0000x reference)
"""AdaptGraphPooling on 8 TRN2 NeuronCores.

Strategy: data-parallel over batch (8 clouds -> 8 cores, fully independent).
Host (numpy) computes the control-flow-heavy, latency-bound parts exactly
mirroring the reference arithmetic (FPS, kNN top-16, index gathers); the
device Bass kernel computes the dense pipeline: pos-MLP, attention MLP
(with BN folded into activation scale/bias), softmax-over-K and the
weighted aggregation -- ~1.6 GFLOP/core on TensorE + DVE/ACT.

Channel-block layout per core (M=1024, K=16, MK=16384, nk-tile=1024):
  conv1:  psum[64,1024] = pw1 @ pos_rel          -> h = Lrelu(s1*psum+b1f)
  attn1:  psum[64,1024] = [W12.T;I64].T @ [h;aq] -> h2 = Lrelu(s2*psum+b2f)
  attn2:  psum[128,1024] x2 + [3,1024]           -> e = Exp(psum + ab2)
  conv2:  psum[128,1024] x2                      -> gf2 = (psum+pb2) + gfeat
  out:    new_feat = sum_k(e*gf2) / sum_k(e)  (per channel,m)
"""

import numpy as np

EPS = 1e-5
B, N, C, D, K, M = 8, 4096, 256, 64, 16, 1024
MK = M * K          # 16384
NT = 32             # outer tiles
NKT = MK // NT      # 512 free elems per tile (32 m x 16 k)
MT = NKT // K       # 64 m rows per tile

_CACHE = {}


# ----------------------------------------------------------------------------
# Host-side exact mirrors of the reference control flow (numpy, float32)
# ----------------------------------------------------------------------------

def _fps_np(xyz):
    """xyz [B,N,3] f32 -> idx [B,M] int64. Bit-exact mirror of reference _fps."""
    dist = np.full((B, N), 1e10, np.float32)
    far = np.zeros((B,), np.int64)
    idxs = np.zeros((B, M), np.int64)
    ar = np.arange(B)
    for t in range(M):
        idxs[:, t] = far
        c = xyz[ar, far]                     # [B,3]
        sq = (xyz - c[:, None, :]) ** 2      # f32
        d = (sq[..., 0] + sq[..., 1]) + sq[..., 2]
        dist = np.minimum(dist, d)
        far = np.argmax(dist, axis=1)        # first occurrence, like jnp.argmax
    return idxs


def _knn_np(xyz, key_xyz):
    """sqr = kk + xx - 2*k.x exactly as reference; stable top-16 by index."""
    sqk = key_xyz ** 2
    kk = (sqk[..., 0] + sqk[..., 1]) + sqk[..., 2]       # [B,M]
    sqx = xyz ** 2
    xx = (sqx[..., 0] + sqx[..., 1]) + sqx[..., 2]       # [B,N]
    dot = np.einsum('bmc,bnc->bmn', key_xyz, xyz).astype(np.float32)
    sqr = (kk[:, :, None] + xx[:, None, :]) - np.float32(2.0) * dot
    # lax.top_k(-sqr, K) is stable (ties -> lowest index): ascending stable sort
    knn = np.argsort(sqr, axis=-1, kind='stable')[..., :K]
    return knn


def _preprocess(inp):
    v = inp['vertices'].astype(np.float32)       # [B,3,N]
    f = inp['feature_map'].astype(np.float32)    # [B,C,N]
    xyz = np.transpose(v, (0, 2, 1)).copy()      # [B,N,3]

    fps_idx = _fps_np(xyz)                       # [B,M]
    ar = np.arange(B)[:, None]
    key_point = np.transpose(xyz[ar, fps_idx], (0, 2, 1))   # [B,3,M]
    key_feat = np.stack([f[b][:, fps_idx[b]] for b in range(B)])  # [B,C,M]
    key_xyz = np.transpose(key_point, (0, 2, 1))             # [B,M,3]

    knn = _knn_np(xyz, key_xyz)                  # [B,M,K]

    group_point = np.stack([v[b][:, knn[b]] for b in range(B)])  # [B,3,M,K]
    group_feat = np.stack([f[b][:, knn[b]] for b in range(B)])   # [B,C,M,K]

    pos_rel = key_point[:, :, :, None] - group_point   # [B,3,M,K]
    qk_rel = key_feat[:, :, :, None] - group_feat      # [B,C,M,K]

    aw1 = inp['aw1'].astype(np.float32)
    aq = np.einsum('dc,bcmk->bdmk', aw1, qk_rel).astype(np.float32)

    return {
        'posrel': pos_rel.reshape(B, 3, MK),
        'aq': aq.reshape(B, D, MK),
        'gfeat': group_feat.reshape(B, C, MK),
        'gpointR': np.ascontiguousarray(
            group_point.reshape(B, 3, 32, 32, K)         # c, t, i, k
            .transpose(0, 1, 2, 3, 4)
            .reshape(B, 3, 32, 32 * K)                   # [B,3,32,512]
            .reshape(B, 3 * 32, 512)),                   # p = c*32+t
    }


def _weights(inp):
    f32 = np.float32
    pw1 = inp['pw1'].astype(f32); pb1 = inp['pb1'].astype(f32)
    s1 = (inp['bn1_g'] / np.sqrt(inp['bn1_v'] + EPS)).astype(f32)
    b1f = (s1 * (pb1 - inp['bn1_m']) + inp['bn1_b']).astype(f32)
    pw2 = inp['pw2'].astype(f32); pb2 = inp['pb2'].astype(f32)
    aw1 = inp['aw1'].astype(f32); ab1 = inp['ab1'].astype(f32)
    s2 = (inp['bn2_g'] / np.sqrt(inp['bn2_v'] + EPS)).astype(f32)
    W12 = (aw1 @ pw2).astype(f32)
    b2f = (s2 * (aw1 @ pb2 + ab1) + (inp['bn2_b'] - inp['bn2_m'] * s2)).astype(f32)
    aw2 = inp['aw2'].astype(f32); ab2 = inp['ab2'].astype(f32)

    # pack every weight into one [128, 652] array; device slices views
    pack = np.zeros((128, 716), f32)
    pack[0:64, 0:64] = W12.T
    pack[0:3, 64:128] = pw1.T
    pack[0:64, 128:256] = aw2[3:131].T
    pack[0:64, 256:384] = aw2[131:259].T
    pack[0:64, 384:512] = pw2[0:128].T
    pack[0:64, 512:640] = pw2[128:256].T
    pack[0:64, 640:643] = aw2[0:3].T
    pack[0:64, 643] = s1
    pack[0:64, 644] = b1f
    pack[0:64, 645] = s2
    pack[0:64, 646] = b2f
    pack[0:128, 647] = ab2[3:131]
    pack[0:128, 648] = ab2[131:259]
    pack[0:128, 649] = pb2[0:128]
    pack[0:128, 650] = pb2[128:256]
    pack[0:3, 651] = ab2[0:3]
    pack[0:64, 652:716] = np.eye(D, dtype=f32)
    return {'wpack': pack}


# ----------------------------------------------------------------------------
# Bass kernel
# ----------------------------------------------------------------------------

def _build():
    import concourse.bass as bass
    import concourse.mybir as mybir
    import concourse.tile as tile
    from concourse import bacc
    from concourse.bass import ts

    f32 = mybir.dt.float32
    AF = mybir.ActivationFunctionType
    ALU = mybir.AluOpType

    nc = bacc.Bacc("TRN2", target_bir_lowering=False)

    # data params
    p_posrel = nc.declare_dram_parameter("posrel", [3, MK], f32, isOutput=False)
    p_aq = nc.declare_dram_parameter("aq", [D, MK], f32, isOutput=False)
    p_gfeat = nc.declare_dram_parameter("gfeat", [C, MK], f32, isOutput=False)
    p_gpr = nc.declare_dram_parameter("gpointR", [96, 512], f32, isOutput=False)
    p_wpack = nc.declare_dram_parameter("wpack", [128, 716], f32, isOutput=False)
    p_out = nc.declare_dram_parameter("out", [3 + C, M], f32, isOutput=True)

    x_scratch = nc.dram_tensor("xscratch", [3, MK], f32)

    with tile.TileContext(nc) as tc:
        with (
            tc.tile_pool(name="wts", bufs=1) as wts,
            tc.tile_pool(name="acc", bufs=1) as acc,
            tc.tile_pool(name="stream", bufs=3) as st,
            tc.tile_pool(name="psum", bufs=1, space="PSUM") as pp,
        ):
            # --- load weights once (single DMA)
            wpk = wts.tile([128, 716], f32)
            nc.sync.dma_start(out=wpk[:], in_=p_wpack[:])
            wt = {
                "WA": wpk[0:64, 0:64], "pw1T": wpk[0:3, 64:128],
                "aw2T_F1": wpk[0:64, 128:256], "aw2T_F2": wpk[0:64, 256:384],
                "pw2T_c1": wpk[0:64, 384:512], "pw2T_c2": wpk[0:64, 512:640],
                "aw2T_X": wpk[0:64, 640:643],
                "s1": wpk[0:64, 643:644], "b1f": wpk[0:64, 644:645],
                "s2": wpk[0:64, 645:646], "b2f": wpk[0:64, 646:647],
                "abF1": wpk[0:128, 647:648], "abF2": wpk[0:128, 648:649],
                "pb2c1": wpk[0:128, 649:650], "pb2c2": wpk[0:128, 650:651],
                "abX": wpk[0:3, 651:652], "I64": wpk[0:64, 652:716],
            }
            gpr = wts.tile([96, 512], f32)
            nc.sync.dma_start(out=gpr[:], in_=p_gpr[:])

            # pre-touch weights on PE so later matmuls carry one less DMA wait
            psD = pp.tile([8, 8], f32, tag="psD")
            nc.tensor.matmul(psD[0:8, 0:8], wpk[0:1, 0:8], wpk[0:1, 0:8],
                             start=True, stop=True)

            # --- accumulators (disjoint m-slices written per tile)
            sum_eA = acc.tile([128, M], f32)
            sum_eB = acc.tile([128, M], f32)
            wsumA = acc.tile([128, M], f32)
            wsumB = acc.tile([128, M], f32)

            for t in range(NT):
                sl = ts(t, NKT)           # free slice in MK
                msl = ts(t, MT)           # m slice

                posrel_t = st.tile([3, NKT], f32, tag="posrel")
                nc.sync.dma_start(out=posrel_t[:], in_=p_posrel[:, sl])
                aq_t = st.tile([D, NKT], f32, tag="aq")
                nc.sync.dma_start(out=aq_t[:], in_=p_aq[:, sl])
                gfA = st.tile([128, NKT], f32, tag="gfA")
                nc.sync.dma_start(out=gfA[:], in_=p_gfeat[0:128, sl])
                gfB = st.tile([128, NKT], f32, tag="gfB")
                nc.sync.dma_start(out=gfB[:], in_=p_gfeat[128:256, sl])

                # conv1 -> h
                ps1 = pp.tile([D, NKT], f32, tag="ps1")
                nc.tensor.matmul(ps1[:], wt["pw1T"], posrel_t[:],
                                 start=True, stop=True)
                h_t = st.tile([D, NKT], f32, tag="h_t")
                nc.scalar.activation(h_t[:], ps1[:], AF.Identity,
                                     bias=wt["b1f"], scale=wt["s1"])
                nc.vector.scalar_tensor_tensor(
                    h_t[:], h_t[:], 0.2, h_t[:], op0=ALU.mult, op1=ALU.max)

                # attn1: W12 @ h + aq -> h2
                ps2 = pp.tile([D, NKT], f32, tag="ps2")
                nc.tensor.matmul(ps2[:], wt["WA"], h_t[:],
                                 start=True, stop=False)
                nc.tensor.matmul(ps2[:], wt["I64"], aq_t[:],
                                 start=False, stop=True)
                h2 = st.tile([D, NKT], f32, tag="h2")
                nc.scalar.activation(h2[:], ps2[:], AF.Identity,
                                     bias=wt["b2f"], scale=wt["s2"])
                nc.vector.scalar_tensor_tensor(
                    h2[:], h2[:], 0.2, h2[:], op0=ALU.mult, op1=ALU.max)

                # conv2 (both chunks into one psum pair) -> gf2 = (pe+pb2)+gfeat
                psP = pp.tile([128, 2 * NKT], f32, tag="psP")
                nc.tensor.matmul(psP[:, 0:NKT], wt["pw2T_c1"], h_t[:],
                                 start=True, stop=True)
                nc.tensor.matmul(psP[:, NKT:2 * NKT], wt["pw2T_c2"], h_t[:],
                                 start=True, stop=True)
                gf2A = st.tile([128, NKT], f32, tag="gf2A")
                gf2B = st.tile([128, NKT], f32, tag="gf2B")
                nc.vector.scalar_tensor_tensor(
                    gf2A[:], psP[:, 0:NKT], wt["pb2c1"], gfA[:],
                    op0=ALU.add, op1=ALU.add)
                nc.vector.scalar_tensor_tensor(
                    gf2B[:], psP[:, NKT:2 * NKT], wt["pb2c2"], gfB[:],
                    op0=ALU.add, op1=ALU.add)

                # attn2 (softmax bias cancels -> no bias) -> e = Exp(logits)
                psF = pp.tile([128, 2 * NKT], f32, tag="psF")
                nc.tensor.matmul(psF[:, 0:NKT], wt["aw2T_F1"], h2[:],
                                 start=True, stop=True)
                nc.tensor.matmul(psF[:, NKT:2 * NKT], wt["aw2T_F2"], h2[:],
                                 start=True, stop=True)
                e = st.tile([128, 2 * NKT], f32, tag="e")
                nc.scalar.activation(e[:], psF[:], AF.Exp)

                for (hh, se, ws, gf2) in ((0, sum_eA, wsumA, gf2A),
                                          (1, sum_eB, wsumB, gf2B)):
                    eh = e[:, ts(hh, NKT)]
                    nc.vector.tensor_reduce(
                        se[:, msl], eh.rearrange("p (m k) -> p m k", k=K),
                        axis=mybir.AxisListType.X, op=ALU.add)
                    nc.vector.tensor_tensor(eh, eh, gf2[:], op=ALU.mult)
                    nc.vector.tensor_reduce(
                        ws[:, msl], eh.rearrange("p (m k) -> p m k", k=K),
                        axis=mybir.AxisListType.X, op=ALU.add)

                # xyz logits -> exp, staged to DRAM for post-loop repack
                psX = pp.tile([3, NKT], f32, tag="psX")
                nc.tensor.matmul(psX[:], wt["aw2T_X"], h2[:],
                                 start=True, stop=True)
                eX_t = st.tile([3, NKT], f32, tag="eX")
                nc.scalar.activation(eX_t[:], psX[:], AF.Exp)
                nc.sync.dma_start(out=x_scratch[:, sl], in_=eX_t[:])

            # --- feature outputs
            for (se, ws, rows) in ((sum_eA, wsumA, (3, 131)),
                                   (sum_eB, wsumB, (131, 259))):
                rec = acc.tile([128, M], f32, tag="rec" + str(rows[0]))
                nc.vector.reciprocal(rec[:], se[:])
                nf = acc.tile([128, M], f32, tag="nf" + str(rows[0]))
                nc.vector.tensor_tensor(nf[:], ws[:], rec[:], op=ALU.mult)
                nc.sync.dma_start(out=p_out[rows[0]:rows[1], :], in_=nf[:])

            # --- xyz path: reload staged exp values repacked to [96,512]
            eXr = acc.tile([96, 512], f32)
            nc.sync.dma_start(
                out=eXr[:],
                in_=x_scratch[:].rearrange("c (u f) -> (c u) f", f=512))
            seX = acc.tile([96, 32], f32)
            nc.vector.tensor_reduce(
                seX[:], eXr[:].rearrange("p (i k) -> p i k", k=K),
                axis=mybir.AxisListType.X, op=ALU.add)
            prX = acc.tile([96, 512], f32)
            nc.vector.tensor_tensor(prX[:], eXr[:], gpr[:], op=ALU.mult)
            wsX = acc.tile([96, 32], f32)
            nc.vector.tensor_reduce(
                wsX[:], prX[:].rearrange("p (i k) -> p i k", k=K),
                axis=mybir.AxisListType.X, op=ALU.add)
            reX = acc.tile([96, 32], f32)
            nc.vector.reciprocal(reX[:], seX[:])
            npX = acc.tile([96, 32], f32)
            nc.vector.tensor_tensor(npX[:], wsX[:], reX[:], op=ALU.mult)
            nc.sync.dma_start(
                out=p_out[0:3, :].rearrange("c (u i) -> (c u) i", i=32),
                in_=npX[:])

    nc.finalize()
    return nc


def kernel(**inputs):
    from concourse.bass_utils import run_bass_kernel_spmd

    data = _preprocess(inputs)
    w = _weights(inputs)

    if 'nc' not in _CACHE:
        _CACHE['nc'] = _build()
    nc = _CACHE['nc']

    in_maps = []
    for b in range(B):
        m = {'posrel': data['posrel'][b], 'aq': data['aq'][b],
             'gfeat': data['gfeat'][b], 'gpointR': data['gpointR'][b]}
        m.update(w)
        in_maps.append(m)

    trace = bool(_CACHE.get('trace'))
    kw = {}
    if trace:
        import sys
        import tempfile
        import types
        if 'antenv.axon_hooks' not in sys.modules:
            import antenv
            mod = types.ModuleType('antenv.axon_hooks')
            mod._hook = None
            def _set(h, _m=mod):
                _m._hook = h
            def _get(_m=mod):
                return _m._hook
            mod.set_axon_ntff_profile_hook = _set
            mod.get_axon_ntff_profile_hook = _get
            sys.modules['antenv.axon_hooks'] = mod
            antenv.axon_hooks = mod
            from trn_agent_boot.trn_boot import _ntff_profile_via_ctypes
            mod.set_axon_ntff_profile_hook(
                _ntff_profile_via_ctypes('/opt/axon/libaxon_pjrt.so'))
        td = tempfile.mkdtemp(prefix='agp_trace_')
        kw = dict(trace=True, tmpdir=td)
        _CACHE['trace_dir'] = td
    res = run_bass_kernel_spmd(nc, in_maps, core_ids=list(range(B)), **kw)
    _CACHE['exec_time_ns'] = getattr(res, 'exec_time_ns', None)
    out = np.stack([res.results[i]['out'] for i in range(B)])
    return out.astype(np.float32)


# revision 25
# speedup vs baseline: 1.4999x; 1.4999x over previous
"""AdaptGraphPooling on 8 TRN2 NeuronCores.

Strategy: data-parallel over batch (8 clouds -> 8 cores, fully independent).
Host (numpy) computes the control-flow-heavy, latency-bound parts exactly
mirroring the reference arithmetic (FPS, kNN top-16, index gathers); the
device Bass kernel computes the dense pipeline: pos-MLP, attention MLP
(with BN folded into activation scale/bias), softmax-over-K and the
weighted aggregation -- ~1.6 GFLOP/core on TensorE + DVE/ACT.

Channel-block layout per core (M=1024, K=16, MK=16384, nk-tile=1024):
  conv1:  psum[64,1024] = pw1 @ pos_rel          -> h = Lrelu(s1*psum+b1f)
  attn1:  psum[64,1024] = [W12.T;I64].T @ [h;aq] -> h2 = Lrelu(s2*psum+b2f)
  attn2:  psum[128,1024] x2 + [3,1024]           -> e = Exp(psum + ab2)
  conv2:  psum[128,1024] x2                      -> gf2 = (psum+pb2) + gfeat
  out:    new_feat = sum_k(e*gf2) / sum_k(e)  (per channel,m)
"""

import numpy as np

EPS = 1e-5
B, N, C, D, K, M = 8, 4096, 256, 64, 16, 1024
MK = M * K          # 16384
NT = 32             # outer tiles
NKT = MK // NT      # 512 free elems per tile (32 m x 16 k)
MT = NKT // K       # 64 m rows per tile

_CACHE = {}


# ----------------------------------------------------------------------------
# Host-side exact mirrors of the reference control flow (numpy, float32)
# ----------------------------------------------------------------------------

def _fps_np(xyz):
    """xyz [B,N,3] f32 -> idx [B,M] int64. Bit-exact mirror of reference _fps."""
    dist = np.full((B, N), 1e10, np.float32)
    far = np.zeros((B,), np.int64)
    idxs = np.zeros((B, M), np.int64)
    ar = np.arange(B)
    for t in range(M):
        idxs[:, t] = far
        c = xyz[ar, far]                     # [B,3]
        sq = (xyz - c[:, None, :]) ** 2      # f32
        d = (sq[..., 0] + sq[..., 1]) + sq[..., 2]
        dist = np.minimum(dist, d)
        far = np.argmax(dist, axis=1)        # first occurrence, like jnp.argmax
    return idxs


def _knn_np(xyz, key_xyz):
    """sqr = kk + xx - 2*k.x exactly as reference; stable top-16 by index."""
    sqk = key_xyz ** 2
    kk = (sqk[..., 0] + sqk[..., 1]) + sqk[..., 2]       # [B,M]
    sqx = xyz ** 2
    xx = (sqx[..., 0] + sqx[..., 1]) + sqx[..., 2]       # [B,N]
    dot = np.einsum('bmc,bnc->bmn', key_xyz, xyz).astype(np.float32)
    sqr = (kk[:, :, None] + xx[:, None, :]) - np.float32(2.0) * dot
    # lax.top_k(-sqr, K) is stable (ties -> lowest index): ascending stable sort
    knn = np.argsort(sqr, axis=-1, kind='stable')[..., :K]
    return knn


def _preprocess(inp):
    v = inp['vertices'].astype(np.float32)       # [B,3,N]
    f = inp['feature_map'].astype(np.float32)    # [B,C,N]
    xyz = np.transpose(v, (0, 2, 1)).copy()      # [B,N,3]

    fps_idx = _fps_np(xyz)                       # [B,M]
    ar = np.arange(B)[:, None]
    key_point = np.transpose(xyz[ar, fps_idx], (0, 2, 1))   # [B,3,M]
    key_feat = np.stack([f[b][:, fps_idx[b]] for b in range(B)])  # [B,C,M]
    key_xyz = np.transpose(key_point, (0, 2, 1))             # [B,M,3]

    knn = _knn_np(xyz, key_xyz)                  # [B,M,K]

    group_point = np.stack([v[b][:, knn[b]] for b in range(B)])  # [B,3,M,K]
    group_feat = np.stack([f[b][:, knn[b]] for b in range(B)])   # [B,C,M,K]

    pos_rel = key_point[:, :, :, None] - group_point   # [B,3,M,K]
    qk_rel = key_feat[:, :, :, None] - group_feat      # [B,C,M,K]

    aw1 = inp['aw1'].astype(np.float32)
    aq = np.einsum('dc,bcmk->bdmk', aw1, qk_rel).astype(np.float32)
    s2 = (inp['bn2_g'] / np.sqrt(inp['bn2_v'] + EPS)).astype(np.float32)
    ab1 = inp['ab1'].astype(np.float32)
    pb2 = inp['pb2'].astype(np.float32)
    b2f = (s2 * (aw1 @ pb2 + ab1)
           + (inp['bn2_b'] - inp['bn2_m'] * s2)).astype(np.float32)
    aqs2 = s2[None, :, None, None] * aq + b2f[None, :, None, None]

    import ml_dtypes
    return {
        'posrel': pos_rel.reshape(B, 3, MK).astype(ml_dtypes.bfloat16),
        'aq': aqs2.reshape(B, D, MK),
        'gfeat': group_feat.reshape(B, C, MK),
        'gpointR': np.ascontiguousarray(
            group_point.reshape(B, 3, 32, 32, K)         # c, t, i, k
            .transpose(0, 1, 2, 3, 4)
            .reshape(B, 3, 32, 32 * K)                   # [B,3,32,512]
            .reshape(B, 3 * 32, 512)),                   # p = c*32+t
    }


def _weights(inp):
    f32 = np.float32
    pw1 = inp['pw1'].astype(f32); pb1 = inp['pb1'].astype(f32)
    s1 = (inp['bn1_g'] / np.sqrt(inp['bn1_v'] + EPS)).astype(f32)
    b1f = (s1 * (pb1 - inp['bn1_m']) + inp['bn1_b']).astype(f32)
    pw2 = inp['pw2'].astype(f32); pb2 = inp['pb2'].astype(f32)
    aw1 = inp['aw1'].astype(f32); ab1 = inp['ab1'].astype(f32)
    s2 = (inp['bn2_g'] / np.sqrt(inp['bn2_v'] + EPS)).astype(f32)
    W12 = (aw1 @ pw2).astype(f32)
    b2f = (s2 * (aw1 @ pb2 + ab1) + (inp['bn2_b'] - inp['bn2_m'] * s2)).astype(f32)
    aw2 = inp['aw2'].astype(f32); ab2 = inp['ab2'].astype(f32)

    # pack every weight into one [128, 652] array; device slices views
    pack = np.zeros((128, 716), f32)
    pack[0:64, 0:64] = W12.T
    pack[0:3, 64:128] = pw1.T
    pack[0:64, 128:256] = aw2[3:131].T
    pack[0:64, 256:384] = aw2[131:259].T
    pack[0:64, 384:512] = pw2[0:128].T
    pack[0:64, 512:640] = pw2[128:256].T
    pack[0:64, 640:643] = aw2[0:3].T
    pack[0:64, 643] = s1
    pack[0:64, 644] = b1f
    pack[0:64, 645] = s2
    pack[0:64, 646] = b2f
    pack[0:128, 647] = ab2[3:131]
    pack[0:128, 648] = ab2[131:259]
    pack[0:128, 649] = pb2[0:128]
    pack[0:128, 650] = pb2[128:256]
    pack[0:3, 651] = ab2[0:3]
    pack[0:64, 652:716] = np.eye(D, dtype=f32)
    import ml_dtypes
    return {'wpack': pack, 'wpackb': pack.astype(ml_dtypes.bfloat16)}


# ----------------------------------------------------------------------------
# Bass kernel
# ----------------------------------------------------------------------------

def _build():
    import concourse.bass as bass
    import concourse.mybir as mybir
    import concourse.tile as tile
    from concourse import bacc
    from concourse.bass import ts

    f32 = mybir.dt.float32
    AF = mybir.ActivationFunctionType
    ALU = mybir.AluOpType

    nc = bacc.Bacc("TRN2", target_bir_lowering=False)

    # data params
    bf16 = mybir.dt.bfloat16
    p_posrel = nc.declare_dram_parameter("posrel", [3, MK], bf16, isOutput=False)
    p_aq = nc.declare_dram_parameter("aq", [D, MK], f32, isOutput=False)
    p_gfeat = nc.declare_dram_parameter("gfeat", [C, MK], f32, isOutput=False)
    p_gpr = nc.declare_dram_parameter("gpointR", [96, 512], f32, isOutput=False)
    p_wpack = nc.declare_dram_parameter("wpack", [128, 716], f32, isOutput=False)
    p_wpackb = nc.declare_dram_parameter("wpackb", [128, 716], bf16, isOutput=False)
    p_out = nc.declare_dram_parameter("out", [3 + C, M], f32, isOutput=True)

    x_scratch = nc.dram_tensor("xscratch", [3, MK], f32)

    with tile.TileContext(nc) as tc:
        with (
            tc.tile_pool(name="wts", bufs=1) as wts,
            tc.tile_pool(name="acc", bufs=1) as acc,
            tc.tile_pool(name="stream", bufs=3) as st,
            tc.tile_pool(name="psum", bufs=1, space="PSUM") as pp,
        ):
            # --- load weights once (single DMA)
            wpk = wts.tile([128, 716], f32)
            nc.sync.dma_start(out=wpk[:], in_=p_wpack[:])
            wpb = wts.tile([128, 716], bf16)
            nc.sync.dma_start(out=wpb[:], in_=p_wpackb[:])
            wt = {
                "WA": wpb[0:64, 0:64], "pw1T": wpb[0:3, 64:128],
                "aw2T_F1": wpb[0:64, 128:256], "aw2T_F2": wpb[0:64, 256:384],
                "pw2T_c1": wpb[0:64, 384:512], "pw2T_c2": wpb[0:64, 512:640],
                "aw2T_X": wpb[0:64, 640:643],
                "s1": wpk[0:64, 643:644], "b1f": wpk[0:64, 644:645],
                "s2": wpk[0:64, 645:646], "b2f": wpk[0:64, 646:647],
                "pb2c1": wpk[0:128, 649:650], "pb2c2": wpk[0:128, 650:651],
            }
            gpr = wts.tile([96, 512], f32)
            nc.sync.dma_start(out=gpr[:], in_=p_gpr[:])

            # pre-touch weights on PE so later matmuls carry one less DMA wait
            psD = pp.tile([8, 8], f32, tag="psD")
            nc.tensor.matmul(psD[0:8, 0:8], wpb[0:1, 0:8], wpb[0:1, 0:8],
                             start=True, stop=True)

            # --- accumulators (disjoint m-slices written per tile)
            sum_eA = acc.tile([128, M], f32)
            sum_eB = acc.tile([128, M], f32)
            wsumA = acc.tile([128, M], f32)
            wsumB = acc.tile([128, M], f32)

            for t in range(NT):
                sl = ts(t, NKT)           # free slice in MK
                msl = ts(t, MT)           # m slice

                posrel_t = st.tile([3, NKT], bf16, tag="posrel")
                nc.sync.dma_start(out=posrel_t[:], in_=p_posrel[:, sl])
                aq_t = st.tile([D, NKT], f32, tag="aq")
                nc.sync.dma_start(out=aq_t[:], in_=p_aq[:, sl])
                gfA = st.tile([128, NKT], f32, tag="gfA")
                nc.sync.dma_start(out=gfA[:], in_=p_gfeat[0:128, sl])
                gfB = st.tile([128, NKT], f32, tag="gfB")
                nc.sync.dma_start(out=gfB[:], in_=p_gfeat[128:256, sl])

                # conv1 -> h
                ps1 = pp.tile([D, NKT], f32, tag="ps1")
                nc.tensor.matmul(ps1[:], wt["pw1T"], posrel_t[:],
                                 start=True, stop=True)
                h_t = st.tile([D, NKT], bf16, tag="h_t")
                nc.scalar.activation(h_t[:], ps1[:], AF.Identity,
                                     bias=wt["b1f"], scale=wt["s1"])
                nc.vector.scalar_tensor_tensor(
                    h_t[:], h_t[:], 0.2, h_t[:], op0=ALU.mult, op1=ALU.max)

                # attn1: z2 = s2*(W12 @ h) + (s2*aq + b2f) -> lrelu -> h2
                ps2 = pp.tile([D, NKT], f32, tag="ps2")
                nc.tensor.matmul(ps2[:], wt["WA"], h_t[:],
                                 start=True, stop=True)
                h2 = st.tile([D, NKT], bf16, tag="h2")
                nc.vector.scalar_tensor_tensor(
                    h2[:], ps2[:], wt["s2"], aq_t[:],
                    op0=ALU.mult, op1=ALU.add)
                nc.vector.scalar_tensor_tensor(
                    h2[:], h2[:], 0.2, h2[:], op0=ALU.mult, op1=ALU.max)

                # conv2 (both chunks into one psum pair) -> gf2 = (pe+pb2)+gfeat
                psP = pp.tile([128, 2 * NKT], f32, tag="psP")
                nc.tensor.matmul(psP[:, 0:NKT], wt["pw2T_c1"], h_t[:],
                                 start=True, stop=True)
                nc.tensor.matmul(psP[:, NKT:2 * NKT], wt["pw2T_c2"], h_t[:],
                                 start=True, stop=True)
                gf2A = st.tile([128, NKT], f32, tag="gf2A")
                gf2B = st.tile([128, NKT], f32, tag="gf2B")
                nc.vector.scalar_tensor_tensor(
                    gf2A[:], psP[:, 0:NKT], wt["pb2c1"], gfA[:],
                    op0=ALU.add, op1=ALU.add)
                nc.vector.scalar_tensor_tensor(
                    gf2B[:], psP[:, NKT:2 * NKT], wt["pb2c2"], gfB[:],
                    op0=ALU.add, op1=ALU.add)

                # attn2 (softmax bias cancels -> no bias) -> e = Exp(logits)
                psF = pp.tile([128, 2 * NKT], f32, tag="psF")
                nc.tensor.matmul(psF[:, 0:NKT], wt["aw2T_F1"], h2[:],
                                 start=True, stop=True)
                nc.tensor.matmul(psF[:, NKT:2 * NKT], wt["aw2T_F2"], h2[:],
                                 start=True, stop=True)
                e = st.tile([128, 2 * NKT], f32, tag="e")
                nc.scalar.activation(e[:], psF[:], AF.Exp)

                for (hh, se, ws, gf2) in ((0, sum_eA, wsumA, gf2A),
                                          (1, sum_eB, wsumB, gf2B)):
                    eh = e[:, ts(hh, NKT)]
                    nc.vector.tensor_reduce(
                        se[:, msl], eh.rearrange("p (m k) -> p m k", k=K),
                        axis=mybir.AxisListType.X, op=ALU.add)
                    nc.vector.tensor_tensor(eh, eh, gf2[:], op=ALU.mult)
                    nc.vector.tensor_reduce(
                        ws[:, msl], eh.rearrange("p (m k) -> p m k", k=K),
                        axis=mybir.AxisListType.X, op=ALU.add)

                # xyz logits -> exp, staged to DRAM for post-loop repack
                psX = pp.tile([3, NKT], f32, tag="psX")
                nc.tensor.matmul(psX[:], wt["aw2T_X"], h2[:],
                                 start=True, stop=True)
                eX_t = st.tile([3, NKT], f32, tag="eX")
                nc.scalar.activation(eX_t[:], psX[:], AF.Exp)
                nc.sync.dma_start(out=x_scratch[:, sl], in_=eX_t[:])

            # --- feature outputs
            for (se, ws, rows) in ((sum_eA, wsumA, (3, 131)),
                                   (sum_eB, wsumB, (131, 259))):
                rec = acc.tile([128, M], f32, tag="rec" + str(rows[0]))
                nc.vector.reciprocal(rec[:], se[:])
                nf = acc.tile([128, M], f32, tag="nf" + str(rows[0]))
                nc.vector.tensor_tensor(nf[:], ws[:], rec[:], op=ALU.mult)
                nc.sync.dma_start(out=p_out[rows[0]:rows[1], :], in_=nf[:])

            # --- xyz path: reload staged exp values repacked to [96,512]
            eXr = acc.tile([96, 512], f32)
            nc.sync.dma_start(
                out=eXr[:],
                in_=x_scratch[:].rearrange("c (u f) -> (c u) f", f=512))
            seX = acc.tile([96, 32], f32)
            nc.vector.tensor_reduce(
                seX[:], eXr[:].rearrange("p (i k) -> p i k", k=K),
                axis=mybir.AxisListType.X, op=ALU.add)
            prX = acc.tile([96, 512], f32)
            nc.vector.tensor_tensor(prX[:], eXr[:], gpr[:], op=ALU.mult)
            wsX = acc.tile([96, 32], f32)
            nc.vector.tensor_reduce(
                wsX[:], prX[:].rearrange("p (i k) -> p i k", k=K),
                axis=mybir.AxisListType.X, op=ALU.add)
            reX = acc.tile([96, 32], f32)
            nc.vector.reciprocal(reX[:], seX[:])
            npX = acc.tile([96, 32], f32)
            nc.vector.tensor_tensor(npX[:], wsX[:], reX[:], op=ALU.mult)
            nc.sync.dma_start(
                out=p_out[0:3, :].rearrange("c (u i) -> (c u) i", i=32),
                in_=npX[:])

    nc.finalize()
    return nc


def kernel(**inputs):
    from concourse.bass_utils import run_bass_kernel_spmd

    data = _preprocess(inputs)
    w = _weights(inputs)

    if 'nc' not in _CACHE:
        _CACHE['nc'] = _build()
    nc = _CACHE['nc']

    in_maps = []
    for b in range(B):
        m = {'posrel': data['posrel'][b], 'aq': data['aq'][b],
             'gfeat': data['gfeat'][b], 'gpointR': data['gpointR'][b]}
        m.update(w)
        in_maps.append(m)

    trace = bool(_CACHE.get('trace'))
    kw = {}
    if trace:
        import sys
        import tempfile
        import types
        if 'antenv.axon_hooks' not in sys.modules:
            import antenv
            mod = types.ModuleType('antenv.axon_hooks')
            mod._hook = None
            def _set(h, _m=mod):
                _m._hook = h
            def _get(_m=mod):
                return _m._hook
            mod.set_axon_ntff_profile_hook = _set
            mod.get_axon_ntff_profile_hook = _get
            sys.modules['antenv.axon_hooks'] = mod
            antenv.axon_hooks = mod
            from trn_agent_boot.trn_boot import _ntff_profile_via_ctypes
            mod.set_axon_ntff_profile_hook(
                _ntff_profile_via_ctypes('/opt/axon/libaxon_pjrt.so'))
        td = tempfile.mkdtemp(prefix='agp_trace_')
        kw = dict(trace=True, tmpdir=td)
        _CACHE['trace_dir'] = td
    res = run_bass_kernel_spmd(nc, in_maps, core_ids=list(range(B)), **kw)
    _CACHE['exec_time_ns'] = getattr(res, 'exec_time_ns', None)
    out = np.stack([res.results[i]['out'] for i in range(B)])
    return out.astype(np.float32)


# revision 26
# speedup vs baseline: 2.0895x; 1.3931x over previous
"""AdaptGraphPooling on 8 TRN2 NeuronCores.

Strategy: data-parallel over batch (8 clouds -> 8 cores, fully independent).
Host (numpy) computes the control-flow-heavy, latency-bound parts exactly
mirroring the reference arithmetic (FPS, kNN top-16, index gathers); the
device Bass kernel computes the dense pipeline: pos-MLP, attention MLP
(with BN folded into matmul weights / activation scale+bias), softmax-over-K
and the weighted aggregation. Matmuls and elementwise products run in bf16;
sums accumulate in fp32; the final division happens on host (exact).

Per-tile flow (nk=512 free = 32 m x 16 k, 32 tiles):
  conv1: ps1[64,512] = pw1b @ posrel              -> h  = Prelu(s1*ps1+b1f)
  attn1: ps2[64,512] = (s2*W12)b @ h + I @ aqs2b  -> h2 = Prelu(ps2)
  conv2: psP[128,1024] = pw2b @ h (2 chunks)      -> gf2 = psP + (gfeat+pb2)
  attn2: psF[128,1024] = aw2b @ h2 (2 chunks)     -> e = Exp(psF)  [bias
         cancels in softmax]; psX[3,512] likewise for the xyz channels
  sums:  sum_e[.,m] += reduce_k e ; wsum[.,m] += reduce_k (e*gf2)
Output: weighted sums and exp-sums; host computes wsum/sum_e.
"""

import numpy as np

EPS = 1e-5
B, N, C, D, K, M = 8, 4096, 256, 64, 16, 1024
MK = M * K          # 16384
NT = 32             # outer tiles
NKT = MK // NT      # 512 free elems per tile (32 m x 16 k)
MT = NKT // K       # 32 m rows per tile

_CACHE = {}


# ----------------------------------------------------------------------------
# Host-side exact mirrors of the reference control flow (numpy, float32)
# ----------------------------------------------------------------------------

def _fps_np(xyz):
    """xyz [B,N,3] f32 -> idx [B,M] int64. Bit-exact mirror of reference _fps."""
    dist = np.full((B, N), 1e10, np.float32)
    far = np.zeros((B,), np.int64)
    idxs = np.zeros((B, M), np.int64)
    ar = np.arange(B)
    for t in range(M):
        idxs[:, t] = far
        c = xyz[ar, far]                     # [B,3]
        sq = (xyz - c[:, None, :]) ** 2      # f32
        d = (sq[..., 0] + sq[..., 1]) + sq[..., 2]
        dist = np.minimum(dist, d)
        far = np.argmax(dist, axis=1)        # first occurrence, like jnp.argmax
    return idxs


def _knn_np(xyz, key_xyz):
    """sqr = kk + xx - 2*k.x exactly as reference; stable top-16 by index."""
    sqk = key_xyz ** 2
    kk = (sqk[..., 0] + sqk[..., 1]) + sqk[..., 2]       # [B,M]
    sqx = xyz ** 2
    xx = (sqx[..., 0] + sqx[..., 1]) + sqx[..., 2]       # [B,N]
    dot = np.einsum('bmc,bnc->bmn', key_xyz, xyz).astype(np.float32)
    sqr = (kk[:, :, None] + xx[:, None, :]) - np.float32(2.0) * dot
    # lax.top_k(-sqr, K) is stable (ties -> lowest index): ascending stable sort
    knn = np.argsort(sqr, axis=-1, kind='stable')[..., :K]
    return knn


def _preprocess(inp):
    import ml_dtypes
    bf = ml_dtypes.bfloat16
    v = inp['vertices'].astype(np.float32)       # [B,3,N]
    f = inp['feature_map'].astype(np.float32)    # [B,C,N]
    xyz = np.transpose(v, (0, 2, 1)).copy()      # [B,N,3]

    fps_idx = _fps_np(xyz)                       # [B,M]
    ar = np.arange(B)[:, None]
    key_point = np.transpose(xyz[ar, fps_idx], (0, 2, 1))   # [B,3,M]
    key_feat = np.stack([f[b][:, fps_idx[b]] for b in range(B)])  # [B,C,M]
    key_xyz = np.transpose(key_point, (0, 2, 1))             # [B,M,3]

    knn = _knn_np(xyz, key_xyz)                  # [B,M,K]

    group_point = np.stack([v[b][:, knn[b]] for b in range(B)])  # [B,3,M,K]
    group_feat = np.stack([f[b][:, knn[b]] for b in range(B)])   # [B,C,M,K]

    pos_rel = key_point[:, :, :, None] - group_point   # [B,3,M,K]
    qk_rel = key_feat[:, :, :, None] - group_feat      # [B,C,M,K]

    aw1 = inp['aw1'].astype(np.float32)
    ab1 = inp['ab1'].astype(np.float32)
    pb2 = inp['pb2'].astype(np.float32)
    s2 = (inp['bn2_g'] / np.sqrt(inp['bn2_v'] + EPS)).astype(np.float32)
    b2f = (s2 * (aw1 @ pb2 + ab1)
           + (inp['bn2_b'] - inp['bn2_m'] * s2)).astype(np.float32)
    aq = np.einsum('dc,bcmk->bdmk', aw1, qk_rel).astype(np.float32)
    aqs2 = s2[None, :, None, None] * aq + b2f[None, :, None, None]

    gfb = group_feat.reshape(B, C, MK) + pb2[None, :, None]

    return {
        'posrel': pos_rel.reshape(B, 3, MK).astype(bf),
        'aq': aqs2.reshape(B, D, MK).astype(bf),
        'gfeat': gfb.astype(bf),
        'gpointR': np.ascontiguousarray(
            group_point.reshape(B, 3, 32, 32, K)
            .reshape(B, 3, 32, 32 * K)
            .reshape(B, 3 * 32, 512)),                   # p = c*32+t
    }


def _weights(inp):
    import ml_dtypes
    f32 = np.float32
    pw1 = inp['pw1'].astype(f32); pb1 = inp['pb1'].astype(f32)
    s1 = (inp['bn1_g'] / np.sqrt(inp['bn1_v'] + EPS)).astype(f32)
    b1f = (s1 * (pb1 - inp['bn1_m']) + inp['bn1_b']).astype(f32)
    pw2 = inp['pw2'].astype(f32)
    aw1 = inp['aw1'].astype(f32)
    s2 = (inp['bn2_g'] / np.sqrt(inp['bn2_v'] + EPS)).astype(f32)
    W12s = (s2[:, None] * (aw1 @ pw2)).astype(f32)
    aw2 = inp['aw2'].astype(f32)

    pack = np.zeros((128, 716), f32)
    pack[0:64, 0:64] = W12s.T
    pack[0:3, 64:128] = pw1.T
    pack[0:64, 128:256] = aw2[3:131].T
    pack[0:64, 256:384] = aw2[131:259].T
    pack[0:64, 384:512] = pw2[0:128].T
    pack[0:64, 512:640] = pw2[128:256].T
    pack[0:64, 640:643] = aw2[0:3].T
    pack[0:64, 643] = s1
    pack[0:64, 644] = b1f
    pack[0:64, 652:716] = np.eye(D, dtype=f32)
    return {'wpack': pack, 'wpackb': pack.astype(ml_dtypes.bfloat16)}


# ----------------------------------------------------------------------------
# Bass kernel
# ----------------------------------------------------------------------------

def _build():
    import concourse.mybir as mybir
    import concourse.tile as tile
    from concourse import bacc
    from concourse.bass import ts

    f32 = mybir.dt.float32
    bf16 = mybir.dt.bfloat16
    AF = mybir.ActivationFunctionType
    ALU = mybir.AluOpType

    nc = bacc.Bacc("TRN2", target_bir_lowering=False)

    p_posrel = nc.declare_dram_parameter("posrel", [3, MK], bf16, isOutput=False)
    p_aq = nc.declare_dram_parameter("aq", [D, MK], bf16, isOutput=False)
    p_gfeat = nc.declare_dram_parameter("gfeat", [C, MK], bf16, isOutput=False)
    p_gpr = nc.declare_dram_parameter("gpointR", [96, 512], f32, isOutput=False)
    p_wpack = nc.declare_dram_parameter("wpack", [128, 716], f32, isOutput=False)
    p_wpackb = nc.declare_dram_parameter("wpackb", [128, 716], bf16,
                                         isOutput=False)
    p_outw = nc.declare_dram_parameter("outw", [3 + C, M], f32, isOutput=True)
    p_oute = nc.declare_dram_parameter("oute", [3 + C, M], f32, isOutput=True)

    x_scratch = nc.dram_tensor("xscratch", [3, MK], f32)

    with tile.TileContext(nc) as tc:
        with (
            tc.tile_pool(name="wts", bufs=1) as wts,
            tc.tile_pool(name="acc", bufs=1) as acc,
            tc.tile_pool(name="stream", bufs=3) as st,
            tc.tile_pool(name="psum", bufs=1, space="PSUM") as pp,
        ):
            wpk = wts.tile([128, 716], f32)
            nc.sync.dma_start(out=wpk[:], in_=p_wpack[:])
            wpb = wts.tile([128, 716], bf16)
            nc.sync.dma_start(out=wpb[:], in_=p_wpackb[:])
            wt = {
                "WA": wpb[0:64, 0:64], "pw1T": wpb[0:3, 64:128],
                "aw2T_F1": wpb[0:64, 128:256], "aw2T_F2": wpb[0:64, 256:384],
                "pw2T_c1": wpb[0:64, 384:512], "pw2T_c2": wpb[0:64, 512:640],
                "aw2T_X": wpb[0:64, 640:643],
                "s1": wpk[0:64, 643:644], "b1f": wpk[0:64, 644:645],
                "I64": wpb[0:64, 652:716],
            }
            gpr = wts.tile([96, 512], f32)
            nc.sync.dma_start(out=gpr[:], in_=p_gpr[:])

            # pre-touch weights on PE so later matmuls carry one less DMA wait
            psD = pp.tile([8, 8], f32, tag="psD")
            nc.tensor.matmul(psD[0:8, 0:8], wpb[0:1, 0:8], wpb[0:1, 0:8],
                             start=True, stop=True)

            # accumulators: interleaved (chunk, m) columns, disjoint per tile
            sum_e2 = acc.tile([128, 2 * M], f32)
            wsum2 = acc.tile([128, 2 * M], f32)

            for t in range(NT):
                sl = ts(t, NKT)

                posrel_t = st.tile([3, NKT], bf16, tag="posrel")
                nc.sync.dma_start(out=posrel_t[:], in_=p_posrel[:, sl])
                aq_t = st.tile([D, NKT], bf16, tag="aq")
                nc.sync.dma_start(out=aq_t[:], in_=p_aq[:, sl])
                gfc = st.tile([128, 2 * NKT], bf16, tag="gfc")
                nc.sync.dma_start(out=gfc[:, 0:NKT], in_=p_gfeat[0:128, sl])
                nc.sync.dma_start(out=gfc[:, NKT:2 * NKT],
                                  in_=p_gfeat[128:256, sl])

                # conv1 -> h = prelu(s1*ps1 + b1f)
                ps1 = pp.tile([D, NKT], f32, tag="ps1")
                nc.tensor.matmul(ps1[:], wt["pw1T"], posrel_t[:],
                                 start=True, stop=True)
                h_t = st.tile([D, NKT], bf16, tag="h_t")
                nc.scalar.activation(h_t[:], ps1[:], AF.Prelu,
                                     bias=wt["b1f"], scale=wt["s1"],
                                     alpha=0.2)

                # attn1: ps2 = (s2*W12) @ h + aqs2 -> h2 = prelu(ps2)
                ps2 = pp.tile([D, NKT], f32, tag="ps2")
                nc.tensor.matmul(ps2[:], wt["WA"], h_t[:],
                                 start=True, stop=False)
                nc.tensor.matmul(ps2[:], wt["I64"], aq_t[:],
                                 start=False, stop=True)
                h2 = st.tile([D, NKT], bf16, tag="h2")
                nc.scalar.activation(h2[:], ps2[:], AF.Prelu, alpha=0.2)

                # conv2 both chunks -> gf2 = psP + (gfeat + pb2)
                psP = pp.tile([128, 2 * NKT], f32, tag="psP")
                nc.tensor.matmul(psP[:, 0:NKT], wt["pw2T_c1"], h_t[:],
                                 start=True, stop=True)
                nc.tensor.matmul(psP[:, NKT:2 * NKT], wt["pw2T_c2"], h_t[:],
                                 start=True, stop=True)
                gf2 = st.tile([128, 2 * NKT], bf16, tag="gf2")
                nc.vector.tensor_tensor(gf2[:], psP[:], gfc[:], op=ALU.add)

                # attn2 both chunks -> e = Exp(logits)
                psF = pp.tile([128, 2 * NKT], f32, tag="psF")
                nc.tensor.matmul(psF[:, 0:NKT], wt["aw2T_F1"], h2[:],
                                 start=True, stop=True)
                nc.tensor.matmul(psF[:, NKT:2 * NKT], wt["aw2T_F2"], h2[:],
                                 start=True, stop=True)
                e = st.tile([128, 2 * NKT], bf16, tag="e")
                nc.scalar.activation(e[:], psF[:], AF.Exp)

                # per-(chunk, m) sums over k
                csl = ts(t, 2 * MT)
                nc.vector.tensor_reduce(
                    sum_e2[:, csl], e[:].rearrange("p (g k) -> p g k", k=K),
                    axis=mybir.AxisListType.X, op=ALU.add)
                nc.vector.tensor_tensor(e[:], e[:], gf2[:], op=ALU.mult)
                nc.vector.tensor_reduce(
                    wsum2[:, csl], e[:].rearrange("p (g k) -> p g k", k=K),
                    axis=mybir.AxisListType.X, op=ALU.add)

                # xyz logits -> exp, staged to DRAM for post-loop repack
                psX = pp.tile([3, NKT], f32, tag="psX")
                nc.tensor.matmul(psX[:], wt["aw2T_X"], h2[:],
                                 start=True, stop=True)
                eX_t = st.tile([3, NKT], f32, tag="eX")
                nc.scalar.activation(eX_t[:], psX[:], AF.Exp)
                nc.sync.dma_start(out=x_scratch[:, sl], in_=eX_t[:])

            # --- feature outputs: strided de-interleave (c even/odd blocks)
            for (src, dst) in ((sum_e2, p_oute), (wsum2, p_outw)):
                s3 = src[:].rearrange("p (t c g) -> p t c g", c=2, g=MT)
                nc.sync.dma_start(
                    out=dst[3:131, :].rearrange("p (t g) -> p t g", g=MT),
                    in_=s3[:, :, 0, :])
                nc.sync.dma_start(
                    out=dst[131:259, :].rearrange("p (t g) -> p t g", g=MT),
                    in_=s3[:, :, 1, :])

            # --- xyz path: reload staged exp values repacked to [96,512]
            eXr = acc.tile([96, 512], f32)
            nc.sync.dma_start(
                out=eXr[:],
                in_=x_scratch[:].rearrange("c (u f) -> (c u) f", f=512))
            seX = acc.tile([96, 32], f32)
            nc.vector.tensor_reduce(
                seX[:], eXr[:].rearrange("p (i k) -> p i k", k=K),
                axis=mybir.AxisListType.X, op=ALU.add)
            nc.vector.tensor_tensor(eXr[:], eXr[:], gpr[:], op=ALU.mult)
            wsX = acc.tile([96, 32], f32)
            nc.vector.tensor_reduce(
                wsX[:], eXr[:].rearrange("p (i k) -> p i k", k=K),
                axis=mybir.AxisListType.X, op=ALU.add)
            nc.sync.dma_start(
                out=p_oute[0:3, :].rearrange("c (u i) -> (c u) i", i=32),
                in_=seX[:])
            nc.sync.dma_start(
                out=p_outw[0:3, :].rearrange("c (u i) -> (c u) i", i=32),
                in_=wsX[:])

    nc.finalize()
    return nc


def kernel(**inputs):
    from concourse.bass_utils import run_bass_kernel_spmd

    data = _preprocess(inputs)
    w = _weights(inputs)

    if 'nc' not in _CACHE:
        _CACHE['nc'] = _build()
    nc = _CACHE['nc']

    in_maps = []
    for b in range(B):
        m = {'posrel': data['posrel'][b], 'aq': data['aq'][b],
             'gfeat': data['gfeat'][b], 'gpointR': data['gpointR'][b]}
        m.update(w)
        in_maps.append(m)

    trace = bool(_CACHE.get('trace'))
    kw = {}
    if trace:
        import sys
        import tempfile
        import types
        if 'antenv.axon_hooks' not in sys.modules:
            import antenv
            mod = types.ModuleType('antenv.axon_hooks')
            mod._hook = None
            def _set(h, _m=mod):
                _m._hook = h
            def _get(_m=mod):
                return _m._hook
            mod.set_axon_ntff_profile_hook = _set
            mod.get_axon_ntff_profile_hook = _get
            sys.modules['antenv.axon_hooks'] = mod
            antenv.axon_hooks = mod
            from trn_agent_boot.trn_boot import _ntff_profile_via_ctypes
            mod.set_axon_ntff_profile_hook(
                _ntff_profile_via_ctypes('/opt/axon/libaxon_pjrt.so'))
        td = tempfile.mkdtemp(prefix='agp_trace_')
        kw = dict(trace=True, tmpdir=td)
        _CACHE['trace_dir'] = td

    res = run_bass_kernel_spmd(nc, in_maps, core_ids=list(range(B)), **kw)
    _CACHE['exec_time_ns'] = getattr(res, 'exec_time_ns', None)
    outw = np.stack([res.results[i]['outw'] for i in range(B)])
    oute = np.stack([res.results[i]['oute'] for i in range(B)])
    return (outw / oute).astype(np.float32)


# revision 30
# speedup vs baseline: 2.4448x; 1.1700x over previous
"""AdaptGraphPooling on 8 TRN2 NeuronCores.

Strategy: data-parallel over batch (8 clouds -> 8 cores, fully independent).
Host (numpy) computes the control-flow-heavy, latency-bound parts exactly
mirroring the reference arithmetic (FPS, kNN top-16, index gathers); the
device Bass kernel computes the dense pipeline: pos-MLP, attention MLP
(with BN folded into matmul weights / activation scale+bias), softmax-over-K
and the weighted aggregation. Matmuls and elementwise products run in bf16;
sums accumulate in fp32; the final division happens on host (exact).

Per-tile flow (nk=512 free = 32 m x 16 k, 32 tiles):
  conv1: ps1[64,512] = pw1b @ posrel              -> h  = Prelu(s1*ps1+b1f)
  attn1: ps2[64,512] = (s2*W12)b @ h + I @ aqs2b  -> h2 = Prelu(ps2)
  conv2: psP[128,1024] = pw2b @ h (2 chunks)      -> gf2 = psP + (gfeat+pb2)
  attn2: psF[128,1024] = aw2b @ h2 (2 chunks)     -> e = Exp(psF)  [bias
         cancels in softmax]; psX[3,512] likewise for the xyz channels
  sums:  sum_e[.,m] += reduce_k e ; wsum[.,m] += reduce_k (e*gf2)
Output: weighted sums and exp-sums; host computes wsum/sum_e.
"""

import numpy as np

EPS = 1e-5
B, N, C, D, K, M = 8, 4096, 256, 64, 16, 1024
MK = M * K          # 16384
NT = 32             # outer tiles
NKT = MK // NT      # 512 free elems per tile (32 m x 16 k)
MT = NKT // K       # 32 m rows per tile

_CACHE = {}


# ----------------------------------------------------------------------------
# Host-side exact mirrors of the reference control flow (numpy, float32)
# ----------------------------------------------------------------------------

def _fps_np(xyz):
    """xyz [B,N,3] f32 -> idx [B,M] int64. Bit-exact mirror of reference _fps."""
    dist = np.full((B, N), 1e10, np.float32)
    far = np.zeros((B,), np.int64)
    idxs = np.zeros((B, M), np.int64)
    ar = np.arange(B)
    for t in range(M):
        idxs[:, t] = far
        c = xyz[ar, far]                     # [B,3]
        sq = (xyz - c[:, None, :]) ** 2      # f32
        d = (sq[..., 0] + sq[..., 1]) + sq[..., 2]
        dist = np.minimum(dist, d)
        far = np.argmax(dist, axis=1)        # first occurrence, like jnp.argmax
    return idxs


def _knn_np(xyz, key_xyz):
    """sqr = kk + xx - 2*k.x exactly as reference; stable top-16 by index."""
    sqk = key_xyz ** 2
    kk = (sqk[..., 0] + sqk[..., 1]) + sqk[..., 2]       # [B,M]
    sqx = xyz ** 2
    xx = (sqx[..., 0] + sqx[..., 1]) + sqx[..., 2]       # [B,N]
    dot = np.einsum('bmc,bnc->bmn', key_xyz, xyz).astype(np.float32)
    sqr = (kk[:, :, None] + xx[:, None, :]) - np.float32(2.0) * dot
    # lax.top_k(-sqr, K) is stable (ties -> lowest index): ascending stable sort
    knn = np.argsort(sqr, axis=-1, kind='stable')[..., :K]
    return knn


def _preprocess(inp):
    import ml_dtypes
    bf = ml_dtypes.bfloat16
    v = inp['vertices'].astype(np.float32)       # [B,3,N]
    f = inp['feature_map'].astype(np.float32)    # [B,C,N]
    xyz = np.transpose(v, (0, 2, 1)).copy()      # [B,N,3]

    fps_idx = _fps_np(xyz)                       # [B,M]
    ar = np.arange(B)[:, None]
    key_point = np.transpose(xyz[ar, fps_idx], (0, 2, 1))   # [B,3,M]
    key_feat = np.stack([f[b][:, fps_idx[b]] for b in range(B)])  # [B,C,M]
    key_xyz = np.transpose(key_point, (0, 2, 1))             # [B,M,3]

    knn = _knn_np(xyz, key_xyz)                  # [B,M,K]

    group_point = np.stack([v[b][:, knn[b]] for b in range(B)])  # [B,3,M,K]
    group_feat = np.stack([f[b][:, knn[b]] for b in range(B)])   # [B,C,M,K]

    pos_rel = key_point[:, :, :, None] - group_point   # [B,3,M,K]
    qk_rel = key_feat[:, :, :, None] - group_feat      # [B,C,M,K]

    aw1 = inp['aw1'].astype(np.float32)
    ab1 = inp['ab1'].astype(np.float32)
    pb2 = inp['pb2'].astype(np.float32)
    s2 = (inp['bn2_g'] / np.sqrt(inp['bn2_v'] + EPS)).astype(np.float32)
    b2f = (s2 * (aw1 @ pb2 + ab1)
           + (inp['bn2_b'] - inp['bn2_m'] * s2)).astype(np.float32)
    aq = np.einsum('dc,bcmk->bdmk', aw1, qk_rel).astype(np.float32)
    aqs2 = s2[None, :, None, None] * aq + b2f[None, :, None, None]

    gfb = group_feat.reshape(B, C, MK) + pb2[None, :, None]

    return {
        'posrel': pos_rel.reshape(B, 3, MK).astype(bf),
        'aq': aqs2.reshape(B, D, MK).astype(bf),
        'gfeat': gfb.astype(bf),
        'gpointR': np.ascontiguousarray(
            group_point.reshape(B, 3, 32, 32, K)
            .reshape(B, 3, 32, 32 * K)
            .reshape(B, 3 * 32, 512)),                   # p = c*32+t
    }


def _weights(inp):
    import ml_dtypes
    f32 = np.float32
    pw1 = inp['pw1'].astype(f32); pb1 = inp['pb1'].astype(f32)
    s1 = (inp['bn1_g'] / np.sqrt(inp['bn1_v'] + EPS)).astype(f32)
    b1f = (s1 * (pb1 - inp['bn1_m']) + inp['bn1_b']).astype(f32)
    pw2 = inp['pw2'].astype(f32)
    aw1 = inp['aw1'].astype(f32)
    s2 = (inp['bn2_g'] / np.sqrt(inp['bn2_v'] + EPS)).astype(f32)
    W12s = (s2[:, None] * (aw1 @ pw2)).astype(f32)
    aw2 = inp['aw2'].astype(f32)

    pack = np.zeros((128, 716), f32)
    pack[0:64, 0:64] = W12s.T
    pack[0:3, 64:128] = pw1.T
    pack[0:64, 128:256] = aw2[3:131].T
    pack[0:64, 256:384] = aw2[131:259].T
    pack[0:64, 384:512] = pw2[0:128].T
    pack[0:64, 512:640] = pw2[128:256].T
    pack[0:64, 640:643] = aw2[0:3].T
    pack[0:64, 643] = s1
    pack[0:64, 644] = b1f
    pack[0:64, 652:716] = np.eye(D, dtype=f32)
    return {'wpack': pack, 'wpackb': pack.astype(ml_dtypes.bfloat16)}


# ----------------------------------------------------------------------------
# Bass kernel
# ----------------------------------------------------------------------------

def _build():
    import concourse.mybir as mybir
    import concourse.tile as tile
    from concourse import bacc
    from concourse.bass import ts

    f32 = mybir.dt.float32
    bf16 = mybir.dt.bfloat16
    AF = mybir.ActivationFunctionType
    ALU = mybir.AluOpType

    nc = bacc.Bacc("TRN2", target_bir_lowering=False)

    p_posrel = nc.declare_dram_parameter("posrel", [3, MK], bf16, isOutput=False)
    p_aq = nc.declare_dram_parameter("aq", [D, MK], bf16, isOutput=False)
    p_gfeat = nc.declare_dram_parameter("gfeat", [C, MK], bf16, isOutput=False)
    p_gpr = nc.declare_dram_parameter("gpointR", [96, 512], f32, isOutput=False)
    p_wpack = nc.declare_dram_parameter("wpack", [128, 716], f32, isOutput=False)
    p_wpackb = nc.declare_dram_parameter("wpackb", [128, 716], bf16,
                                         isOutput=False)
    p_outw = nc.declare_dram_parameter("outw", [3 + C, M], f32, isOutput=True)
    p_oute = nc.declare_dram_parameter("oute", [3 + C, M], f32, isOutput=True)

    x_scratch = nc.dram_tensor("xscratch", [3, MK], f32)

    with tile.TileContext(nc) as tc:
        with (
            tc.tile_pool(name="wts", bufs=1) as wts,
            tc.tile_pool(name="acc", bufs=1) as acc,
            tc.tile_pool(name="stream", bufs=3) as st,
            tc.tile_pool(name="psum", bufs=1, space="PSUM") as pp,
        ):
            wpk = wts.tile([128, 716], f32)
            nc.sync.dma_start(out=wpk[:], in_=p_wpack[:])
            wpb = wts.tile([128, 716], bf16)
            nc.sync.dma_start(out=wpb[:], in_=p_wpackb[:])
            wt = {
                "WA": wpb[0:64, 0:64], "pw1T": wpb[0:3, 64:128],
                "aw2T_F1": wpb[0:64, 128:256], "aw2T_F2": wpb[0:64, 256:384],
                "pw2T_c1": wpb[0:64, 384:512], "pw2T_c2": wpb[0:64, 512:640],
                "aw2T_X": wpb[0:64, 640:643],
                "s1": wpk[0:64, 643:644], "b1f": wpk[0:64, 644:645],
                "I64": wpb[0:64, 652:716],
            }
            gpr = wts.tile([96, 512], f32)
            nc.sync.dma_start(out=gpr[:], in_=p_gpr[:])

            # accumulators: interleaved (chunk, m) columns, disjoint per tile
            sum_e2 = acc.tile([128, 2 * M], f32)
            wsum2 = acc.tile([128, 2 * M], f32)

            for t in range(NT):
                sl = ts(t, NKT)

                posrel_t = st.tile([3, NKT], bf16, tag="posrel")
                nc.sync.dma_start(out=posrel_t[:], in_=p_posrel[:, sl])
                aq_t = st.tile([D, NKT], bf16, tag="aq")
                nc.sync.dma_start(out=aq_t[:], in_=p_aq[:, sl])
                gfc = st.tile([128, 2 * NKT], bf16, tag="gfc")
                nc.sync.dma_start(out=gfc[:, 0:NKT], in_=p_gfeat[0:128, sl])
                nc.sync.dma_start(out=gfc[:, NKT:2 * NKT],
                                  in_=p_gfeat[128:256, sl])

                # conv1 -> h = prelu(s1*ps1 + b1f)
                ps1 = pp.tile([D, NKT], f32, tag="ps12x", bufs=2)
                nc.tensor.matmul(ps1[:], wt["pw1T"], posrel_t[:],
                                 start=True, stop=True)
                h_t = st.tile([D, NKT], bf16, tag="h_t")
                nc.scalar.activation(h_t[:], ps1[:], AF.Prelu,
                                     bias=wt["b1f"], scale=wt["s1"],
                                     alpha=0.2)

                # attn1: ps2 = (s2*W12) @ h + aqs2 -> h2 = prelu(ps2)
                ps2 = pp.tile([D, NKT], f32, tag="ps12x", bufs=2)
                nc.tensor.matmul(ps2[:], wt["WA"], h_t[:],
                                 start=True, stop=False)
                nc.tensor.matmul(ps2[:], wt["I64"], aq_t[:],
                                 start=False, stop=True)

                # conv2 both chunks (independent of h2 -> keep PE streaming)
                psP = pp.tile([128, 2 * NKT], f32, tag="psP", bufs=2)
                nc.tensor.matmul(psP[:, 0:NKT], wt["pw2T_c1"], h_t[:],
                                 start=True, stop=True)
                nc.tensor.matmul(psP[:, NKT:2 * NKT], wt["pw2T_c2"], h_t[:],
                                 start=True, stop=True)

                h2 = st.tile([D, NKT], bf16, tag="h2")
                nc.scalar.activation(h2[:], ps2[:], AF.Prelu, alpha=0.2)

                gf2 = st.tile([128, 2 * NKT], bf16, tag="gf2")
                nc.vector.tensor_tensor(gf2[:], psP[:], gfc[:], op=ALU.add)

                # attn2 both chunks -> e = Exp(logits)
                psF = pp.tile([128, 2 * NKT], f32, tag="psF", bufs=1)
                nc.tensor.matmul(psF[:, 0:NKT], wt["aw2T_F1"], h2[:],
                                 start=True, stop=True)
                nc.tensor.matmul(psF[:, NKT:2 * NKT], wt["aw2T_F2"], h2[:],
                                 start=True, stop=True)
                e = st.tile([128, 2 * NKT], bf16, tag="e")
                nc.scalar.activation(e[:], psF[:], AF.Exp)

                # per-(chunk, m) sums over k
                csl = ts(t, 2 * MT)
                nc.vector.tensor_reduce(
                    sum_e2[:, csl], e[:].rearrange("p (g k) -> p g k", k=K),
                    axis=mybir.AxisListType.X, op=ALU.add)
                nc.vector.tensor_tensor(e[:], e[:], gf2[:], op=ALU.mult)
                nc.vector.tensor_reduce(
                    wsum2[:, csl], e[:].rearrange("p (g k) -> p g k", k=K),
                    axis=mybir.AxisListType.X, op=ALU.add)

                # xyz logits -> exp, staged to DRAM for post-loop repack
                psX = pp.tile([3, NKT], f32, tag="ps12x", bufs=2)
                nc.tensor.matmul(psX[:], wt["aw2T_X"], h2[:],
                                 start=True, stop=True)
                eX_t = st.tile([3, NKT], f32, tag="eX")
                nc.scalar.activation(eX_t[:], psX[:], AF.Exp)
                nc.sync.dma_start(out=x_scratch[:, sl], in_=eX_t[:])

            # --- feature outputs: strided de-interleave (c even/odd blocks)
            for (src, dst) in ((sum_e2, p_oute), (wsum2, p_outw)):
                s3 = src[:].rearrange("p (t c g) -> p t c g", c=2, g=MT)
                nc.sync.dma_start(
                    out=dst[3:131, :].rearrange("p (t g) -> p t g", g=MT),
                    in_=s3[:, :, 0, :])
                nc.sync.dma_start(
                    out=dst[131:259, :].rearrange("p (t g) -> p t g", g=MT),
                    in_=s3[:, :, 1, :])

            # --- xyz path: reload staged exp values repacked to [96,512]
            eXr = acc.tile([96, 512], f32)
            nc.sync.dma_start(
                out=eXr[:],
                in_=x_scratch[:].rearrange("c (u f) -> (c u) f", f=512))
            seX = acc.tile([96, 32], f32)
            nc.vector.tensor_reduce(
                seX[:], eXr[:].rearrange("p (i k) -> p i k", k=K),
                axis=mybir.AxisListType.X, op=ALU.add)
            nc.vector.tensor_tensor(eXr[:], eXr[:], gpr[:], op=ALU.mult)
            wsX = acc.tile([96, 32], f32)
            nc.vector.tensor_reduce(
                wsX[:], eXr[:].rearrange("p (i k) -> p i k", k=K),
                axis=mybir.AxisListType.X, op=ALU.add)
            nc.sync.dma_start(
                out=p_oute[0:3, :].rearrange("c (u i) -> (c u) i", i=32),
                in_=seX[:])
            nc.sync.dma_start(
                out=p_outw[0:3, :].rearrange("c (u i) -> (c u) i", i=32),
                in_=wsX[:])

    nc.finalize()
    return nc


def kernel(**inputs):
    from concourse.bass_utils import run_bass_kernel_spmd

    data = _preprocess(inputs)
    w = _weights(inputs)

    if 'nc' not in _CACHE:
        _CACHE['nc'] = _build()
    nc = _CACHE['nc']

    in_maps = []
    for b in range(B):
        m = {'posrel': data['posrel'][b], 'aq': data['aq'][b],
             'gfeat': data['gfeat'][b], 'gpointR': data['gpointR'][b]}
        m.update(w)
        in_maps.append(m)

    trace = bool(_CACHE.get('trace'))
    kw = {}
    if trace:
        import sys
        import tempfile
        import types
        if 'antenv.axon_hooks' not in sys.modules:
            import antenv
            mod = types.ModuleType('antenv.axon_hooks')
            mod._hook = None
            def _set(h, _m=mod):
                _m._hook = h
            def _get(_m=mod):
                return _m._hook
            mod.set_axon_ntff_profile_hook = _set
            mod.get_axon_ntff_profile_hook = _get
            sys.modules['antenv.axon_hooks'] = mod
            antenv.axon_hooks = mod
            from trn_agent_boot.trn_boot import _ntff_profile_via_ctypes
            mod.set_axon_ntff_profile_hook(
                _ntff_profile_via_ctypes('/opt/axon/libaxon_pjrt.so'))
        td = tempfile.mkdtemp(prefix='agp_trace_')
        kw = dict(trace=True, tmpdir=td)
        _CACHE['trace_dir'] = td

    res = run_bass_kernel_spmd(nc, in_maps, core_ids=list(range(B)), **kw)
    _CACHE['exec_time_ns'] = getattr(res, 'exec_time_ns', None)
    outw = np.stack([res.results[i]['outw'] for i in range(B)])
    oute = np.stack([res.results[i]['oute'] for i in range(B)])
    return (outw / oute).astype(np.float32)
